# revision 15
# baseline (speedup 1.0000x reference)
"""FCOS loss on 8 TRN2 NeuronCores — data-parallel over the batch dim.

Per core (1 image) the FCOS target assignment is computed WITHOUT any
[P, M] = 21504x32 pairwise tensor work on the vector engines:

  * The per-(point,box) validity test is separable per axis:
      valid = Px(x,m)*Py(y,m) - Qx(x,m)*Qy(y,m)
    where Px/Qx are tiny [64, grid] indicator matrices built from the box
    coords (P = inside & below-hi, Q = P & below-lo).
  * Boxes are pre-sorted by area (host, stable), so argmin-by-area = first
    valid box.  c = sum_m 4^-m * valid is computed by the TensorEngine as an
    indicator matmul; the f32 EXPONENT of c yields m0 exactly.
  * Winner payloads (quantized box coords + label) come from wide fp32r
    matmuls (payload-major moving operand, >=256 cols -> 1 cycle/row) with
    weights 2^(-16*(m&7)) * payload gated per 8-box range; t =
    S[range(m0)] * 2^(16*(m0&7)) = payload + tail (tail<0.5), so an int
    truncation recovers the quantized payload.

Focal / GIoU / centerness losses are computed densely (bf16 where 2x/4x DVE
modes apply), spread across DVE / Activation / Pool engines, and reduced to
six partial sums per core; the host combines the 8 cores' partials.
"""
import sys

for _p in ("/opt/trn_rl_repo", "/root/.axon_site/_ro/trn_rl_repo"):
    if _p not in sys.path:
        sys.path.insert(0, _p)

import numpy as np

import concourse.bass as bass
import concourse.tile as tile
from concourse.tile_rust import add_dep_helper
from concourse import bacc, mybir
from concourse.bass_utils import run_bass_kernel_spmd

DT = mybir.dt
ALU = mybir.AluOpType
AF = mybir.ActivationFunctionType
AX = mybir.AxisListType

# ---------------- static problem constants ----------------
NCLS = 20
M = 32
NPTS = 21504
G = 168                      # point chunks of 128
STRIDES = [4, 8, 16]
LVLW = [128, 64, 32]         # per-level grid width (= height)
LVLXO = [0, 128, 192]        # offset of level's grid slice in the 224 axis
LVLGO = [0, 128, 160]        # offset of level's chunks in the G axis
GW = 224
LOGIT03 = -0.8472978603872036  # log(0.3/0.7): p>0.3  <=>  x>logit(0.3)


def _static_consts():
    grid = np.concatenate([
        (np.arange(w, dtype=np.float32) * s + s / 2.0).astype(np.float32)
        for w, s in zip(LVLW, STRIDES)
    ])
    grid128 = np.tile(grid[None, :], (128, 1)).astype(np.float32)

    # x-major flat order per level: f = x*h + y  ->  p = f%128, g = f//128
    xsys = np.zeros((128, 2, G), np.float32)
    for lvl, (w, s) in enumerate(zip(LVLW, STRIDES)):
        gvals = (np.arange(w, dtype=np.float32) * s + s / 2.0).astype(np.float32)
        npts = w * w
        flat = np.arange(npts)
        x, y = flat // w, flat % w
        p = flat % 128
        g = LVLGO[lvl] + flat // 128
        xsys[p, 0, g] = gvals[x]
        xsys[p, 1, g] = gvals[y]
    return grid128, xsys


GRID_C, XSYS_C = _static_consts()
import ml_dtypes as _mld
_BF16 = _mld.bfloat16
IOTAX_C = np.ascontiguousarray(
    np.broadcast_to(np.arange(NCLS, dtype=np.float32)[None, :, None], (128, NCLS, G))
).astype(_BF16)


def _prep_image(boxes, labels):
    """Per-image host prep: sorted-box scalars + weight tables."""
    boxes = np.asarray(boxes, np.float32)
    labels = np.asarray(labels)
    areas = (boxes[:, 2] - boxes[:, 0]) * (boxes[:, 3] - boxes[:, 1])
    order = np.argsort(areas, kind="stable")
    b = boxes[order]
    lab = labels[order].astype(np.float32)
    x0, y0, x1, y1 = b[:, 0], b[:, 1], b[:, 2], b[:, 3]
    gq = np.stack([
        np.round(x0 * 32.0), np.round(y0 * 32.0),
        np.round(x1 * 32.0), np.round(y1 * 32.0),
        lab * 32.0,
    ]).astype(np.float64)                      # [5, M]

    ks = np.arange(64)
    ms = ks >> 1
    sgn = np.where((ks & 1) == 1, -1.0, 1.0)   # pq=1 rows carry -Q

    scal = np.zeros((128, 8), np.float32)
    scal[0:64, 0] = -x0[ms]
    scal[64:128, 0] = -y0[ms]
    scal[0:64, 1] = x1[ms]
    scal[64:128, 1] = y1[ms]
    scal[0:64, 2] = (sgn * np.exp2(-2.0 * ms)).astype(np.float32)   # +-4^-m
    scal[0:64, 3] = (ks & 1).astype(np.float32)
    scal[64:128, 3] = (ks & 1).astype(np.float32)
    scal[:, 4] = 1.0

    wallt = np.zeros((64, 20), np.float32)
    for pay in range(5):
        for r in range(4):
            col = pay * 4 + r
            sel = (ms >> 3) == r
            w = sgn * np.exp2(-16.0 * (ms & 7)) * gq[pay, ms]
            wallt[sel, col] = w[sel].astype(np.float32)
    return scal, wallt


_CACHE = {}


def _build():
    if "nc" in _CACHE:
        return _CACHE["nc"]
    nc = bacc.Bacc("TRN2", target_bir_lowering=False, debug=False)

    cls_d = nc.dram_tensor("cls", [128, NCLS, G], DT.bfloat16, kind="ExternalInput")
    iotax_d = nc.dram_tensor("iotax", [128, NCLS, G], DT.bfloat16, kind="ExternalInput")
    reg_d = nc.dram_tensor("reg", [128, 5, G], DT.bfloat16, kind="ExternalInput")
    cst_d = nc.dram_tensor("cst", [128, 624], DT.float32, kind="ExternalInput")
    out_d = nc.dram_tensor("out", [128, 8], DT.float32, kind="ExternalOutput")

    F32, I32, BF, F32R = DT.float32, DT.int32, DT.bfloat16, DT.float32r
    with tile.TileContext(nc) as tc:
        with (
            tc.tile_pool(name="cst", bufs=1) as cst,
            tc.tile_pool(name="wk", bufs=1) as wk,
            tc.tile_pool(name="ps", bufs=1, space="PSUM") as psp,
        ):
            CST = cst.tile([128, 624], F32)
            nc.sync.dma_start(CST[:], cst_d.ap())
            GRID = CST[:, 0:224]
            XSYS = CST[:, 224:560].rearrange("p (a g) -> p a g", a=2)
            SCAL = CST[:, 580:588]
            WALLT = CST[0:64, 588:608]

            CLS = wk.tile([128, NCLS, G], BF)
            nc.scalar.dma_start(CLS[:], cls_d.ap())
            REGC = wk.tile([128, 5, G], BF)
            nc.scalar.dma_start(REGC[:], reg_d.ap())
            IOTAX = wk.tile([128, NCLS, G], BF)
            nc.sync.dma_start(IOTAX[:], iotax_d.ap())
            REG = REGC[:, 0:4, :]
            CTRP = REGC[:, 4, :]
            BIAS635 = CST[:, 616:617]
            BIASM7 = CST[:, 617:618]
            BIASM15 = CST[:, 618:619]
            BIASM23 = CST[:, 619:620]

            # dummy sigmoid first so the initial (free) act-table load is the
            # sigmoid set; everything else in phase 1 (Identity/Sign/Copy/
            # Square) is present in every set.
            DUM = wk.tile([128, 1], F32)
            i_dum = nc.scalar.activation(DUM[:], GRID[:, 0:1], AF.Sigmoid)

            # ---------------- indicator construction ----------------
            # rows 0:64 = x-side (k = 2m+pq), rows 64:128 = y-side
            TL = wk.tile([128, GW], F32)
            TR = wk.tile([128, GW], F32)
            MN = wk.tile([128, GW], F32)
            MXT = wk.tile([128, GW], F32)
            AIN = wk.tile([128, GW], F32)
            PT = wk.tile([128, GW], F32)
            QT = wk.tile([128, GW], F32)
            DQ = wk.tile([128, GW], F32)
            PQ = wk.tile([128, GW], F32)
            i_tl = nc.scalar.activation(TL[:], GRID, AF.Identity, bias=SCAL[:, 0:1], scale=1.0)
            nc.scalar.activation(TR[:], GRID, AF.Identity, bias=SCAL[:, 1:2], scale=-1.0)
            add_dep_helper(i_tl.ins, i_dum.ins, sync=False, reason="act order")
            nc.vector.tensor_tensor(out=MN[:], in0=TL[:], in1=TR[:], op=ALU.min)
            nc.vector.tensor_tensor(out=MXT[:], in0=TL[:], in1=TR[:], op=ALU.max)
            nc.vector.tensor_scalar(out=AIN[:], in0=MN[:], scalar1=0.0, scalar2=None, op0=ALU.is_gt)
            # P = inside & (mx <= hi)   (level 2: hi = inf)
            nc.vector.scalar_tensor_tensor(
                out=PT[:, 0:128], in0=MXT[:, 0:128], scalar=64.0, in1=AIN[:, 0:128],
                op0=ALU.is_le, op1=ALU.mult)
            nc.vector.scalar_tensor_tensor(
                out=PT[:, 128:192], in0=MXT[:, 128:192], scalar=128.0, in1=AIN[:, 128:192],
                op0=ALU.is_le, op1=ALU.mult)
            nc.vector.tensor_copy(PT[:, 192:224], AIN[:, 192:224])
            # Q = P & (mx < lo)          (level 0: lo = -1 -> Q = 0)
            nc.gpsimd.memset(QT[:, 0:128], 0.0)
            nc.vector.scalar_tensor_tensor(
                out=QT[:, 128:192], in0=MXT[:, 128:192], scalar=64.0, in1=PT[:, 128:192],
                op0=ALU.is_lt, op1=ALU.mult)
            nc.vector.scalar_tensor_tensor(
                out=QT[:, 192:224], in0=MXT[:, 192:224], scalar=128.0, in1=PT[:, 192:224],
                op0=ALU.is_lt, op1=ALU.mult)
            # blend rows by pq parity: PQ = P + pqm*(Q-P)
            nc.gpsimd.tensor_tensor(out=DQ[:], in0=QT[:], in1=PT[:], op=ALU.subtract)
            nc.vector.scalar_tensor_tensor(
                out=PQ[:], in0=DQ[:], scalar=SCAL[:, 3:4], in1=PT[:],
                op0=ALU.mult, op1=ALU.add)

            YSD = wk.tile([64, GW], F32R)
            LCR = wk.tile([64, GW], F32R)
            MEGA = wk.tile([64, 20, GW], F32R)
            nc.vector.tensor_copy(YSD[:], PQ[64:128, :])
            nc.vector.tensor_scalar(out=LCR[:], in0=PQ[0:64, :], scalar1=SCAL[0:64, 2:3],
                                    scalar2=None, op0=ALU.mult)
            nc.vector.tensor_tensor(
                out=MEGA[:, 0:10, :],
                in0=PQ[0:64, :].unsqueeze(1).broadcast_to([64, 10, GW]),
                in1=WALLT[:, 0:10].unsqueeze(2).broadcast_to([64, 10, GW]),
                op=ALU.mult)
            nc.gpsimd.tensor_tensor(
                out=MEGA[:, 10:20, :],
                in0=PQ[0:64, :].unsqueeze(1).broadcast_to([64, 10, GW]),
                in1=WALLT[:, 10:20].unsqueeze(2).broadcast_to([64, 10, GW]),
                op=ALU.mult)

            # ---------------- per-level matmuls + extraction ----------------
            # transposed layout: out[y, ., x];  lhsT = YSD (stationary)
            POS = wk.tile([128, G], BF)
            PVA = wk.tile([128, 5, G], I32)

            for lvl in range(3):
                W = LVLW[lvl]
                xs = slice(LVLXO[lvl], LVLXO[lvl] + W)
                cps = psp.tile([W, W], F32, tag="cps", name="cps")
                sps = psp.tile([W, 5, 4, W], F32, tag="sps", name="sps")
                nc.tensor.matmul(cps[:], YSD[:, xs], LCR[:, xs], start=True, stop=True)
                if lvl < 2:
                    for pay in range(5):
                        nc.tensor.matmul(
                            sps[:, pay, :, :],
                            YSD[:, xs],
                            MEGA[:, 4 * pay:4 * pay + 4, xs],
                            start=True, stop=True)
                else:
                    for p2 in range(2):
                        nc.tensor.matmul(
                            sps[:, 2 * p2:2 * p2 + 2, :, :],
                            YSD[:, xs],
                            MEGA[:, 8 * p2:8 * p2 + 8, xs],
                            start=True, stop=True)
                    nc.tensor.matmul(
                        sps[:, 4, :, :], YSD[:, xs],
                        MEGA[:, 16:20, xs],
                        start=True, stop=True)

                if lvl == 0:
                    posl = POS[:, 0:128]
                else:
                    posl_t = wk.tile([W, W], BF, tag=f"posl{lvl}", name=f"posl{lvl}")
                    posl = posl_t[:]
                nc.scalar.sign(posl, cps[:])
                # m0 extraction: int bit-chain on DVE, is_ge range masks on Pool
                EI = wk.tile([W, W], I32, tag=f"ei{lvl}", name=f"ei{lvl}")
                M0F = wk.tile([W, W], F32, tag=f"m0f{lvl}", name=f"m0f{lvl}")
                I0 = wk.tile([W, W], I32, tag=f"i0{lvl}", name=f"i0{lvl}")
                SCB = wk.tile([W, W], I32, tag=f"scb{lvl}", name=f"scb{lvl}")
                nc.vector.tensor_scalar(out=EI[:], in0=cps[:].bitcast(I32),
                                        scalar1=23, scalar2=None, op0=ALU.arith_shift_right)
                nc.vector.tensor_scalar(out=M0F[:], in0=EI[:], scalar1=-0.5, scalar2=63.5,
                                        op0=ALU.mult, op1=ALU.add)
                nc.vector.tensor_copy(I0[:], M0F[:])
                nc.vector.tensor_scalar(out=I0[:], in0=I0[:], scalar1=7, scalar2=None,
                                        op0=ALU.bitwise_and)
                nc.vector.tensor_scalar(out=SCB[:], in0=I0[:], scalar1=27, scalar2=None,
                                        op0=ALU.logical_shift_left)
                nc.vector.tensor_scalar(out=SCB[:], in0=SCB[:], scalar1=127 << 23, scalar2=None,
                                        op0=ALU.add)
                MG8 = wk.tile([W, W], I32, tag=f"mg8{lvl}", name=f"mg8{lvl}")
                MG16 = wk.tile([W, W], I32, tag=f"mg16{lvl}", name=f"mg16{lvl}")
                MG24 = wk.tile([W, W], I32, tag=f"mg24{lvl}", name=f"mg24{lvl}")
                nc.vector.tensor_scalar(out=MG8[:], in0=M0F[:], scalar1=8.0, scalar2=None, op0=ALU.is_ge)
                nc.vector.tensor_scalar(out=MG16[:], in0=M0F[:], scalar1=16.0, scalar2=None, op0=ALU.is_ge)
                nc.vector.tensor_scalar(out=MG24[:], in0=M0F[:], scalar1=24.0, scalar2=None, op0=ALU.is_ge)
                TSEL = wk.tile([W, 5, W + 4], F32, tag=f"tsel{lvl}", name=f"tsel{lvl}")
                tsl = TSEL[:, :, 0:W]
                nc.scalar.copy(tsl, sps[:, :, 0, :])
                nc.vector.copy_predicated(tsl, MG8[:].unsqueeze(1).broadcast_to([W, 5, W]), sps[:, :, 1, :])
                nc.vector.copy_predicated(tsl, MG16[:].unsqueeze(1).broadcast_to([W, 5, W]), sps[:, :, 2, :])
                nc.vector.copy_predicated(tsl, MG24[:].unsqueeze(1).broadcast_to([W, 5, W]), sps[:, :, 3, :])
                nc.vector.tensor_tensor(
                    out=tsl, in0=tsl,
                    in1=SCB[:].bitcast(F32).unsqueeze(1).broadcast_to([W, 5, W]),
                    op=ALU.mult)
                GI = wk.tile([W, 5, W], I32, tag=f"gi{lvl}", name=f"gi{lvl}")
                if lvl == 0:
                    nc.vector.tensor_copy(PVA[:, :, 0:128], tsl)
                elif lvl == 1:
                    nc.vector.tensor_copy(GI[:], tsl)
                    gv = GI[:].rearrange("p q (g two) -> p q two g", two=2)
                    pv = posl.rearrange("p (g two) -> p two g", two=2)
                    nc.scalar.copy(PVA[0:64, :, 128:160], gv[:, :, 0, :])
                    nc.scalar.copy(PVA[64:128, :, 128:160], gv[:, :, 1, :])
                    nc.scalar.copy(POS[0:64, 128:160], pv[:, 0, :])
                    nc.scalar.copy(POS[64:128, 128:160], pv[:, 1, :])
                else:
                    nc.vector.tensor_copy(GI[:], tsl)
                    gv = GI[:].rearrange("p q (g four) -> p q four g", four=4)
                    pv = posl.rearrange("p (g four) -> p four g", four=4)
                    for j in range(4):
                        nc.gpsimd.tensor_copy(PVA[32 * j:32 * j + 32, :, 160:168], gv[:, :, j, :])
                        nc.gpsimd.tensor_copy(POS[32 * j:32 * j + 32, 160:168], pv[:, j, :])

            # ---------------- per-point targets (bf16, on Pool) ----------------
            TGT = wk.tile([128, 4, G], BF)
            nc.vector.scalar_tensor_tensor(
                out=TGT[:, 0:2, :], in0=PVA[:, 0:2, :], scalar=-0.03125, in1=XSYS,
                op0=ALU.mult, op1=ALU.add)
            nc.vector.scalar_tensor_tensor(
                out=TGT[:, 2:4, :], in0=PVA[:, 2:4, :], scalar=0.03125, in1=XSYS,
                op0=ALU.mult, op1=ALU.subtract)

            ACC = wk.tile([128, 8], F32)
            nc.gpsimd.memset(ACC[:], 0.0)

            # ---------------- dense focal (bf16) ----------------
            SGN = wk.tile([128, NCLS, G], BF)
            SP = wk.tile([128, NCLS, G], BF)
            SQ = wk.tile([128, NCLS, G], BF)
            BW = wk.tile([128, NCLS, G], BF)
            i_sgn = nc.scalar.activation(SGN[:], CLS[:], AF.Sigmoid, scale=-1.0)
            # centerness bce sigmoid (phase 1, input ready early)
            SPC = wk.tile([128, G], F32)
            i_bcesig = nc.scalar.activation(SPC[:], CTRP, AF.Sigmoid, scale=-1.0)
            add_dep_helper(i_bcesig.ins, i_sgn.ins, sync=False, reason="act order")
            # ---- ln phase 1 ----
            i_spln = nc.scalar.activation(SP[:], SGN[:], AF.Ln)      # -softplus(x)
            add_dep_helper(i_spln.ins, i_bcesig.ins, sync=False, reason="act order")
            i_sq = nc.scalar.activation(SQ[:], SGN[:], AF.Square, scale=-1.0, bias=1.0)
            i_bceln = nc.scalar.activation(SPC[:], SPC[:], AF.Ln)    # -softplus(ctr)
            BASE = SQ  # reuse buffer: BASE = SP * SQ
            nc.vector.tensor_tensor(out=BASE[:], in0=SP[:], in1=SQ[:], op=ALU.mult)

            # max over classes -> w-mask in logit space (Pool tree)
            MT10 = wk.tile([128, 10, G], BF)
            MT5 = wk.tile([128, 5, G], BF)
            MXL = wk.tile([128, G], BF)
            nc.vector.tensor_tensor(out=MT10[:], in0=CLS[:, 0:10, :], in1=CLS[:, 10:20, :], op=ALU.max)
            nc.vector.tensor_tensor(out=MT5[:], in0=MT10[:, 0:5, :], in1=MT10[:, 5:10, :], op=ALU.max)
            nc.vector.tensor_tensor(out=MT10[:, 0:2, :], in0=MT5[:, 0:2, :], in1=MT5[:, 2:4, :], op=ALU.max)
            nc.vector.tensor_tensor(out=MT10[:, 2:3, :], in0=MT10[:, 0:1, :], in1=MT10[:, 1:2, :], op=ALU.max)
            nc.vector.tensor_tensor(out=MXL[:].unsqueeze(1), in0=MT10[:, 2:3, :], in1=MT5[:, 4:5, :], op=ALU.max)
            HIM = wk.tile([128, G], BF)
            WBAR = wk.tile([128, G], BF)
            W16 = wk.tile([128, G], BF)
            nc.vector.tensor_scalar(out=HIM[:], in0=MXL[:], scalar1=LOGIT03, scalar2=None, op0=ALU.is_gt)
            nc.vector.tensor_scalar(out=WBAR[:], in0=POS[:], scalar1=-1.0, scalar2=1.0,
                                    op0=ALU.mult, op1=ALU.add)
            nc.vector.tensor_tensor(out=WBAR[:], in0=WBAR[:], in1=HIM[:], op=ALU.mult)
            nc.vector.tensor_scalar(out=W16[:], in0=WBAR[:], scalar1=-0.75, scalar2=0.75,
                                    op0=ALU.mult, op1=ALU.add)   # 0.75*w
            nc.vector.tensor_tensor(out=BW[:], in0=BASE[:],
                                    in1=W16[:].unsqueeze(1).broadcast_to([128, NCLS, G]),
                                    op=ALU.mult)                 # -base*w
            # PE: sum over all elements of -base*w
            ABP = psp.tile([1, 512], F32, tag="abp", name="abp")
            bw = BW[:].rearrange("p c g -> p (c g)")
            for i in range(7):
                n0 = i * 512
                n1 = min(n0 + 512, NCLS * G)
                nc.tensor.matmul(ABP[0:1, 0:n1 - n0], CST[:, 608:609].bitcast(BF)[:, 0:1],
                                 bw[:, n0:n1], start=(i == 0), stop=(i == 6))
            ABSB = wk.tile([1, 512], F32)
            i_absb = nc.scalar.copy(ABSB[:], ABP[:])
            nc.vector.tensor_reduce(out=ACC[0:1, 6:7], in_=ABSB[:], axis=AX.X, op=ALU.add)

            # ---------------- label-column logit ----------------
            LAB16 = wk.tile([128, G], BF)
            nc.vector.tensor_scalar(out=LAB16[:], in0=PVA[:, 4, :], scalar1=0.03125,
                                    scalar2=None, op0=ALU.mult)
            ISEQ = wk.tile([128, NCLS, G], BF)
            XLP = wk.tile([128, NCLS, G], BF)
            nc.vector.tensor_tensor(
                out=ISEQ[:], in0=LAB16[:].unsqueeze(1).broadcast_to([128, NCLS, G]),
                in1=IOTAX[:], op=ALU.is_equal)
            nc.vector.tensor_tensor(out=XLP[:], in0=ISEQ[:], in1=CLS[:], op=ALU.mult)
            XT10 = wk.tile([128, 10, G], BF)
            XT5 = wk.tile([128, 5, G], BF)
            XL = wk.tile([128, G], F32)
            nc.vector.tensor_tensor(out=XT10[:], in0=XLP[:, 0:10, :], in1=XLP[:, 10:20, :], op=ALU.add)
            nc.vector.tensor_tensor(out=XT5[:], in0=XT10[:, 0:5, :], in1=XT10[:, 5:10, :], op=ALU.add)
            nc.vector.tensor_tensor(out=XT10[:, 0:2, :], in0=XT5[:, 0:2, :], in1=XT5[:, 2:4, :], op=ALU.add)
            nc.vector.tensor_tensor(out=XT10[:, 2:3, :], in0=XT10[:, 0:1, :], in1=XT10[:, 1:2, :], op=ALU.add)
            nc.vector.tensor_tensor(out=XL[:].unsqueeze(1), in0=XT10[:, 2:3, :], in1=XT5[:, 4:5, :], op=ALU.add)
            # ---- sigmoid phase 2 ----
            SGL = wk.tile([128, G], F32)
            SGLN = wk.tile([128, G], F32)
            i_sgl = nc.scalar.activation(SGL[:], XL[:], AF.Sigmoid)
            i_sgln = nc.scalar.activation(SGLN[:], XL[:], AF.Sigmoid, scale=-1.0)
            add_dep_helper(i_sgl.ins, i_bceln.ins, sync=False, reason="act order")
            # ---- ln phase 2 ----
            SPL = wk.tile([128, G], F32)
            i_spl = nc.scalar.activation(SPL[:], SGL[:], AF.Ln)     # = -softplus(-xl)
            add_dep_helper(i_spl.ins, i_sgln.ins, sync=False, reason="act order")
            i_sgln2 = nc.scalar.activation(SGLN[:], SGLN[:], AF.Ln)  # = -softplus(xl)
            add_dep_helper(i_absb.ins, i_sgln2.ins, sync=False, reason="act tail order")
            # precompute pos-weighted squares while the ln table loads, then
            # each of D1/D2 is a single fused stt-accumulate (host subtracts)
            B1 = wk.tile([128, G], F32)
            SG2 = wk.tile([128, G], F32)
            D1 = wk.tile([128, G], F32)
            D2 = wk.tile([128, G], F32)
            nc.vector.tensor_scalar(out=B1[:], in0=SGL[:], scalar1=-1.0, scalar2=1.0,
                                    op0=ALU.mult, op1=ALU.add)
            nc.vector.tensor_tensor(out=B1[:], in0=B1[:], in1=B1[:], op=ALU.mult)
            nc.vector.tensor_tensor(out=B1[:], in0=B1[:], in1=POS[:], op=ALU.mult)
            nc.vector.tensor_tensor(out=SG2[:], in0=SGL[:], in1=SGL[:], op=ALU.mult)
            nc.vector.tensor_tensor(out=SG2[:], in0=SG2[:], in1=POS[:], op=ALU.mult)
            nc.vector.scalar_tensor_tensor(out=D1[:], in0=SPL[:], scalar=-0.25, in1=B1[:],
                                           op0=ALU.mult, op1=ALU.mult, accum_out=ACC[:, 5:6])
            nc.vector.scalar_tensor_tensor(out=D2[:], in0=SGLN[:], scalar=-0.75, in1=SG2[:],
                                           op0=ALU.mult, op1=ALU.mult, accum_out=ACC[:, 7:8])

            # ---------------- GIoU (raw, bf16, clamped denominators) ----------
            MINS = wk.tile([128, 4, G], BF)
            MAXS = wk.tile([128, 4, G], BF)
            nc.vector.tensor_tensor(out=MINS[:], in0=REG, in1=TGT[:], op=ALU.min)
            nc.vector.tensor_tensor(out=MAXS[:], in0=REG, in1=TGT[:], op=ALU.max)
            SUMP = wk.tile([128, 2, G], BF)
            SUMT = wk.tile([128, 2, G], BF)
            WIHI = wk.tile([128, 2, G], BF)
            GWGH = wk.tile([128, 2, G], BF)
            nc.vector.tensor_tensor(out=SUMP[:], in0=REG[:, 0:2, :], in1=REG[:, 2:4, :], op=ALU.add)
            nc.gpsimd.tensor_tensor(out=SUMT[:], in0=TGT[:, 0:2, :], in1=TGT[:, 2:4, :], op=ALU.add)
            nc.vector.tensor_tensor(out=WIHI[:], in0=MINS[:, 0:2, :], in1=MINS[:, 2:4, :], op=ALU.add)
            nc.vector.tensor_tensor(out=GWGH[:], in0=MAXS[:, 0:2, :], in1=MAXS[:, 2:4, :], op=ALU.add)
            PAREA = wk.tile([128, G], BF)
            TAREA = wk.tile([128, G], BF)
            AI = wk.tile([128, G], BF)
            ACX = wk.tile([128, G], BF)
            nc.vector.tensor_tensor(out=PAREA[:], in0=SUMP[:, 0, :], in1=SUMP[:, 1, :], op=ALU.mult)
            nc.vector.tensor_tensor(out=TAREA[:], in0=SUMT[:, 0, :], in1=SUMT[:, 1, :], op=ALU.mult)
            nc.vector.tensor_tensor(out=AI[:], in0=WIHI[:, 0, :], in1=WIHI[:, 1, :], op=ALU.mult)
            nc.vector.tensor_tensor(out=ACX[:], in0=GWGH[:, 0, :], in1=GWGH[:, 1, :], op=ALU.mult)
            AU = wk.tile([128, G], BF)
            AU1 = wk.tile([128, G], F32)
            nc.vector.tensor_tensor(out=AU[:], in0=TAREA[:], in1=PAREA[:], op=ALU.add)
            nc.vector.tensor_tensor(out=AU[:], in0=AU[:], in1=AI[:], op=ALU.subtract)
            nc.vector.tensor_scalar(out=AU1[:], in0=AU[:], scalar1=1.0, scalar2=None, op0=ALU.add)
            # clamp |au+1| away from 0 (sign-free: max with eps keeps finiteness)
            nc.vector.tensor_scalar(out=AU1[:], in0=AU1[:], scalar1=1e-10, scalar2=None, op0=ALU.max)
            RAU = wk.tile([128, G], F32)
            IOUS = wk.tile([128, G], F32)
            nc.vector.reciprocal(RAU[:], AU1[:])
            nc.vector.tensor_scalar(out=IOUS[:], in0=AI[:], scalar1=1.0, scalar2=None, op0=ALU.add)
            nc.vector.tensor_tensor(out=IOUS[:], in0=IOUS[:], in1=RAU[:], op=ALU.mult)
            ACXC = wk.tile([128, G], F32)
            RAC = wk.tile([128, G], F32)
            T3 = wk.tile([128, G], F32)
            nc.vector.tensor_scalar(out=ACXC[:], in0=ACX[:], scalar1=1e-10, scalar2=None, op0=ALU.max)
            nc.vector.reciprocal(RAC[:], ACXC[:])
            nc.vector.tensor_tensor(out=T3[:], in0=ACX[:], in1=AU[:], op=ALU.subtract)
            nc.gpsimd.tensor_tensor(out=T3[:], in0=T3[:], in1=RAC[:], op=ALU.mult)
            LB = wk.tile([128, G], F32)
            # lb = 1 - ious + t3 (the +1 folds into the final stt below)
            nc.gpsimd.tensor_tensor(out=LB[:], in0=T3[:], in1=IOUS[:], op=ALU.subtract)

            # centerness target from raw tgt: sqrt via rsqrt magic (no table)
            LRMIN = wk.tile([128, G], BF)
            LRMAX = wk.tile([128, G], BF)
            TBMIN = wk.tile([128, G], BF)
            TBMAX = wk.tile([128, G], BF)
            nc.vector.tensor_tensor(out=LRMIN[:], in0=TGT[:, 0, :], in1=TGT[:, 2, :], op=ALU.min)
            nc.vector.tensor_tensor(out=LRMAX[:], in0=TGT[:, 0, :], in1=TGT[:, 2, :], op=ALU.max)
            nc.vector.tensor_tensor(out=TBMIN[:], in0=TGT[:, 1, :], in1=TGT[:, 3, :], op=ALU.min)
            nc.vector.tensor_tensor(out=TBMAX[:], in0=TGT[:, 1, :], in1=TGT[:, 3, :], op=ALU.max)
            NUMR = wk.tile([128, G], F32)
            DEN = wk.tile([128, G], F32)
            Z = wk.tile([128, G], F32)
            nc.gpsimd.tensor_tensor(out=NUMR[:], in0=LRMIN[:], in1=TBMIN[:], op=ALU.mult)
            nc.gpsimd.tensor_tensor(out=DEN[:], in0=LRMAX[:], in1=TBMAX[:], op=ALU.mult)
            nc.vector.tensor_scalar(out=NUMR[:], in0=NUMR[:], scalar1=0.0, scalar2=None, op0=ALU.max)
            nc.vector.tensor_scalar(out=DEN[:], in0=DEN[:], scalar1=1e-12, scalar2=None, op0=ALU.max)
            nc.gpsimd.tensor_tensor(out=Z[:], in0=NUMR[:], in1=DEN[:], op=ALU.mult)
            nc.vector.tensor_scalar(out=Z[:], in0=Z[:], scalar1=1e-20, scalar2=None, op0=ALU.max)
            # ctr = numr * rsqrt(z) = sqrt(numr/den); one Newton step
            R0 = wk.tile([128, G], I32)
            R2 = wk.tile([128, G], F32)
            V = wk.tile([128, G], F32)
            CTRT = wk.tile([128, G], F32)
            nc.vector.tensor_scalar(out=R0[:], in0=Z[:].bitcast(I32), scalar1=1,
                                    scalar2=None, op0=ALU.logical_shift_right)
            nc.vector.tensor_scalar(out=R0[:], in0=R0[:], scalar1=-1, scalar2=0x5f3759df,
                                    op0=ALU.mult, op1=ALU.add)
            R0F = R0[:].bitcast(F32)
            nc.gpsimd.tensor_tensor(out=R2[:], in0=R0F, in1=R0F, op=ALU.mult)
            nc.gpsimd.tensor_tensor(out=R2[:], in0=R2[:], in1=Z[:], op=ALU.mult)
            nc.gpsimd.tensor_scalar(out=V[:], in0=R2[:], scalar1=-0.5, scalar2=1.5,
                                    op0=ALU.mult, op1=ALU.add)
            nc.gpsimd.tensor_tensor(out=V[:], in0=V[:], in1=R0F, op=ALU.mult)
            nc.gpsimd.tensor_tensor(out=CTRT[:], in0=NUMR[:], in1=V[:], op=ALU.mult)
            W2 = wk.tile([128, G], F32)
            nc.gpsimd.tensor_tensor(out=W2[:], in0=CTRT[:], in1=POS[:], op=ALU.mult)
            LBW = wk.tile([128, G], F32)
            nc.vector.scalar_tensor_tensor(out=LBW[:], in0=LB[:], scalar=1.0, in1=W2[:],
                                           op0=ALU.add, op1=ALU.mult, accum_out=ACC[:, 2:3])
            # centerness bce
            UC = wk.tile([128, G], F32)
            nc.vector.tensor_tensor(out=UC[:], in0=CTRP, in1=CTRT[:], op=ALU.mult)
            nc.vector.tensor_tensor(out=UC[:], in0=SPC[:], in1=UC[:], op=ALU.add)
            VCP = wk.tile([128, G], F32)
            nc.vector.scalar_tensor_tensor(out=VCP[:], in0=UC[:], scalar=-1.0, in1=POS[:],
                                           op0=ALU.mult, op1=ALU.mult, accum_out=ACC[:, 3:4])
            # num_pos
            PCP = wk.tile([128, G], F32)
            nc.vector.tensor_scalar(out=PCP[:], in0=POS[:], scalar1=1.0, scalar2=0.0,
                                    op0=ALU.mult, op1=ALU.add, accum_out=ACC[:, 4:5])

            # ---------------- finalize: ship raw per-partition accumulators;
            # the host does the 128-way column sum (cheaper than a matmul +
            # copy + tiny DMA on the critical tail)
            nc.sync.dma_start(out_d.ap(), ACC[:])

    nc.compile()
    _CACHE["nc"] = nc
    return nc


def make_in_map(cls_l, reg_l, ctr_l, boxes, labels):
    """Build one core's input map from per-image numpy arrays."""
    scal, wallt = _prep_image(boxes, labels)

    def xmaj(p):
        # [C, h, w] -> x-major flat [C, w*h]
        c = p.shape[0]
        return p.transpose(0, 2, 1).reshape(c, -1)

    cls_cat = np.concatenate([xmaj(p.reshape(NCLS, p.shape[-2], p.shape[-1]))
                              for p in cls_l], 1)
    reg_cat = np.concatenate([xmaj(p.reshape(4, p.shape[-2], p.shape[-1]))
                              for p in reg_l], 1)
    ctr_cat = np.concatenate([xmaj(p.reshape(1, p.shape[-2], p.shape[-1]))
                              for p in ctr_l], 1)
    # partition-major repack: [C, (g p)] -> [p, C, g]
    cls_pm = cls_cat.reshape(NCLS, G, 128).transpose(2, 0, 1)
    regc = np.concatenate([reg_cat, ctr_cat], 0)
    reg_pm = regc.reshape(5, G, 128).transpose(2, 0, 1)
    cst = np.zeros((128, 624), np.float32)
    cst[:, 0:224] = GRID_C
    cst[:, 224:560] = XSYS_C.reshape(128, 336)
    cst[:, 580:588] = scal
    cst[0:64, 588:608] = wallt
    ones2 = np.full((128, 2), 1.0, _BF16)
    cst[:, 608:609] = ones2.view(np.float32)
    cst[:, 616] = 63.5
    cst[:, 617] = -7.0
    cst[:, 618] = -15.0
    cst[:, 619] = -23.0
    return {
        "cls": np.ascontiguousarray(cls_pm).astype(_BF16),
        "iotax": IOTAX_C,
        "reg": np.ascontiguousarray(reg_pm).astype(_BF16),
        "cst": cst,
    }


def combine_partials(parts):
    """parts: [n_cores, 8] -> [3] losses."""
    s = np.asarray(parts, np.float64).sum(0)
    Cv, D, E, F, ABn = s[2], s[3], s[4], s[5] - s[7], s[6]
    np_ = max(E, 1.0)
    return np.array([(-ABn + F) / np_, Cv / np_, D / np_], np.float32)


def kernel(cls0, cls1, cls2, reg0, reg1, reg2, ctr0, ctr1, ctr2, boxes, labels,
           _trace=False):
    nc = _build()
    B = np.asarray(boxes).shape[0]
    in_maps = []
    for i in range(B):
        in_maps.append(make_in_map(
            [np.asarray(cls0)[i], np.asarray(cls1)[i], np.asarray(cls2)[i]],
            [np.asarray(reg0)[i], np.asarray(reg1)[i], np.asarray(reg2)[i]],
            [np.asarray(ctr0)[i], np.asarray(ctr1)[i], np.asarray(ctr2)[i]],
            np.asarray(boxes)[i], np.asarray(labels)[i]))
    res = run_bass_kernel_spmd(nc, in_maps, core_ids=list(range(B)), trace=_trace)
    parts = [r["out"].astype(np.float64).sum(axis=0) for r in res.results]
    out = combine_partials(parts)
    if _trace:
        return out, res
    return out


# revision 16
# speedup vs baseline: 1.0057x; 1.0057x over previous
"""FCOS loss on 8 TRN2 NeuronCores — data-parallel over the batch dim.

Per core (1 image) the FCOS target assignment is computed WITHOUT any
[P, M] = 21504x32 pairwise tensor work on the vector engines:

  * The per-(point,box) validity test is separable per axis:
      valid = Px(x,m)*Py(y,m) - Qx(x,m)*Qy(y,m)
    where Px/Qx are tiny [64, grid] indicator matrices built from the box
    coords (P = inside & below-hi, Q = P & below-lo).
  * Boxes are pre-sorted by area (host, stable), so argmin-by-area = first
    valid box.  c = sum_m 4^-m * valid is computed by the TensorEngine as an
    indicator matmul; the f32 EXPONENT of c yields m0 exactly.
  * Winner payloads (quantized box coords + label) come from wide fp32r
    matmuls (payload-major moving operand, >=256 cols -> 1 cycle/row) with
    weights 2^(-16*(m&7)) * payload gated per 8-box range; t =
    S[range(m0)] * 2^(16*(m0&7)) = payload + tail (tail<0.5), so an int
    truncation recovers the quantized payload.

Focal / GIoU / centerness losses are computed densely (bf16 where 2x/4x DVE
modes apply), spread across DVE / Activation / Pool engines, and reduced to
six partial sums per core; the host combines the 8 cores' partials.
"""
import sys

for _p in ("/opt/trn_rl_repo", "/root/.axon_site/_ro/trn_rl_repo"):
    if _p not in sys.path:
        sys.path.insert(0, _p)

import numpy as np

import concourse.bass as bass
import concourse.tile as tile
from concourse.tile_rust import add_dep_helper
from concourse import bacc, mybir
from concourse.bass_utils import run_bass_kernel_spmd

DT = mybir.dt
ALU = mybir.AluOpType
AF = mybir.ActivationFunctionType
AX = mybir.AxisListType

# ---------------- static problem constants ----------------
NCLS = 20
M = 32
NPTS = 21504
G = 168                      # point chunks of 128
STRIDES = [4, 8, 16]
LVLW = [128, 64, 32]         # per-level grid width (= height)
LVLXO = [0, 128, 192]        # offset of level's grid slice in the 224 axis
LVLGO = [0, 128, 160]        # offset of level's chunks in the G axis
GW = 224
LOGIT03 = -0.8472978603872036  # log(0.3/0.7): p>0.3  <=>  x>logit(0.3)


def _static_consts():
    grid = np.concatenate([
        (np.arange(w, dtype=np.float32) * s + s / 2.0).astype(np.float32)
        for w, s in zip(LVLW, STRIDES)
    ])
    grid128 = np.tile(grid[None, :], (128, 1)).astype(np.float32)

    # x-major flat order per level: f = x*h + y  ->  p = f%128, g = f//128
    xsys = np.zeros((128, 2, G), np.float32)
    for lvl, (w, s) in enumerate(zip(LVLW, STRIDES)):
        gvals = (np.arange(w, dtype=np.float32) * s + s / 2.0).astype(np.float32)
        npts = w * w
        flat = np.arange(npts)
        x, y = flat // w, flat % w
        p = flat % 128
        g = LVLGO[lvl] + flat // 128
        xsys[p, 0, g] = gvals[x]
        xsys[p, 1, g] = gvals[y]
    return grid128, xsys


GRID_C, XSYS_C = _static_consts()
import ml_dtypes as _mld
_BF16 = _mld.bfloat16
IOTAX_C = np.ascontiguousarray(
    np.broadcast_to(np.arange(NCLS, dtype=np.float32)[None, :, None], (128, NCLS, G))
).astype(_BF16)


def _prep_image(boxes, labels):
    """Per-image host prep: sorted-box scalars + weight tables."""
    boxes = np.asarray(boxes, np.float32)
    labels = np.asarray(labels)
    areas = (boxes[:, 2] - boxes[:, 0]) * (boxes[:, 3] - boxes[:, 1])
    order = np.argsort(areas, kind="stable")
    b = boxes[order]
    lab = labels[order].astype(np.float32)
    x0, y0, x1, y1 = b[:, 0], b[:, 1], b[:, 2], b[:, 3]
    gq = np.stack([
        np.round(x0 * 32.0), np.round(y0 * 32.0),
        np.round(x1 * 32.0), np.round(y1 * 32.0),
        lab * 32.0,
    ]).astype(np.float64)                      # [5, M]

    ks = np.arange(64)
    ms = ks >> 1
    sgn = np.where((ks & 1) == 1, -1.0, 1.0)   # pq=1 rows carry -Q

    scal = np.zeros((128, 8), np.float32)
    scal[0:64, 0] = -x0[ms]
    scal[64:128, 0] = -y0[ms]
    scal[0:64, 1] = x1[ms]
    scal[64:128, 1] = y1[ms]
    scal[0:64, 2] = (sgn * np.exp2(-2.0 * ms)).astype(np.float32)   # +-4^-m
    scal[0:64, 3] = (ks & 1).astype(np.float32)
    scal[64:128, 3] = (ks & 1).astype(np.float32)
    scal[:, 4] = 1.0

    wallt = np.zeros((64, 20), np.float32)
    for pay in range(5):
        for r in range(4):
            col = pay * 4 + r
            sel = (ms >> 3) == r
            w = sgn * np.exp2(-16.0 * (ms & 7)) * gq[pay, ms]
            wallt[sel, col] = w[sel].astype(np.float32)
    return scal, wallt


_CACHE = {}


def _build():
    if "nc" in _CACHE:
        return _CACHE["nc"]
    nc = bacc.Bacc("TRN2", target_bir_lowering=False, debug=False)

    cls_d = nc.dram_tensor("cls", [128, NCLS, G], DT.bfloat16, kind="ExternalInput")
    iotax_d = nc.dram_tensor("iotax", [128, NCLS, G], DT.bfloat16, kind="ExternalInput")
    reg_d = nc.dram_tensor("reg", [128, 5, G], DT.bfloat16, kind="ExternalInput")
    cst_d = nc.dram_tensor("cst", [128, 624], DT.float32, kind="ExternalInput")
    out_d = nc.dram_tensor("out", [128, 8], DT.float32, kind="ExternalOutput")

    F32, I32, BF, F32R = DT.float32, DT.int32, DT.bfloat16, DT.float32r
    with tile.TileContext(nc) as tc:
        with (
            tc.tile_pool(name="cst", bufs=1) as cst,
            tc.tile_pool(name="wk", bufs=1) as wk,
            tc.tile_pool(name="ps", bufs=1, space="PSUM") as psp,
        ):
            CST = cst.tile([128, 624], F32)
            nc.sync.dma_start(CST[:], cst_d.ap())
            GRID = CST[:, 0:224]
            XSYS = CST[:, 224:560].rearrange("p (a g) -> p a g", a=2)
            SCAL = CST[:, 580:588]
            WALLT = CST[0:64, 588:608]

            CLS = wk.tile([128, NCLS, G], BF)
            nc.scalar.dma_start(CLS[:], cls_d.ap())
            REGC = wk.tile([128, 5, G], BF)
            nc.scalar.dma_start(REGC[:], reg_d.ap())
            IOTAX = wk.tile([128, NCLS, G], BF)
            nc.sync.dma_start(IOTAX[:], iotax_d.ap())
            REG = REGC[:, 0:4, :]
            CTRP = REGC[:, 4, :]
            BIAS635 = CST[:, 616:617]
            BIASM7 = CST[:, 617:618]
            BIASM15 = CST[:, 618:619]
            BIASM23 = CST[:, 619:620]

            # dummy sigmoid first so the initial (free) act-table load is the
            # sigmoid set; everything else in phase 1 (Identity/Sign/Copy/
            # Square) is present in every set.
            DUM = wk.tile([128, 1], F32)
            i_dum = nc.scalar.activation(DUM[:], GRID[:, 0:1], AF.Sigmoid)

            # ---------------- indicator construction ----------------
            # rows 0:64 = x-side (k = 2m+pq), rows 64:128 = y-side
            TL = wk.tile([128, GW], F32)
            TR = wk.tile([128, GW], F32)
            MN = wk.tile([128, GW], F32)
            MXT = wk.tile([128, GW], F32)
            AIN = wk.tile([128, GW], F32)
            PT = wk.tile([128, GW], F32)
            QT = wk.tile([128, GW], F32)
            DQ = wk.tile([128, GW], F32)
            PQ = wk.tile([128, GW], F32)
            i_tl = nc.scalar.activation(TL[:], GRID, AF.Identity, bias=SCAL[:, 0:1], scale=1.0)
            nc.scalar.activation(TR[:], GRID, AF.Identity, bias=SCAL[:, 1:2], scale=-1.0)
            add_dep_helper(i_tl.ins, i_dum.ins, sync=False, reason="act order")
            nc.vector.tensor_tensor(out=MN[:], in0=TL[:], in1=TR[:], op=ALU.min)
            nc.vector.tensor_tensor(out=MXT[:], in0=TL[:], in1=TR[:], op=ALU.max)
            nc.vector.tensor_scalar(out=AIN[:], in0=MN[:], scalar1=0.0, scalar2=None, op0=ALU.is_gt)
            # P = inside & (mx <= hi)   (level 2: hi = inf)
            nc.vector.scalar_tensor_tensor(
                out=PT[:, 0:128], in0=MXT[:, 0:128], scalar=64.0, in1=AIN[:, 0:128],
                op0=ALU.is_le, op1=ALU.mult)
            nc.vector.scalar_tensor_tensor(
                out=PT[:, 128:192], in0=MXT[:, 128:192], scalar=128.0, in1=AIN[:, 128:192],
                op0=ALU.is_le, op1=ALU.mult)
            nc.vector.tensor_copy(PT[:, 192:224], AIN[:, 192:224])
            # Q = P & (mx < lo)          (level 0: lo = -1 -> Q = 0)
            nc.gpsimd.memset(QT[:, 0:128], 0.0)
            nc.vector.scalar_tensor_tensor(
                out=QT[:, 128:192], in0=MXT[:, 128:192], scalar=64.0, in1=PT[:, 128:192],
                op0=ALU.is_lt, op1=ALU.mult)
            nc.vector.scalar_tensor_tensor(
                out=QT[:, 192:224], in0=MXT[:, 192:224], scalar=128.0, in1=PT[:, 192:224],
                op0=ALU.is_lt, op1=ALU.mult)
            # blend rows by pq parity: PQ = P + pqm*(Q-P)
            nc.gpsimd.tensor_tensor(out=DQ[:], in0=QT[:], in1=PT[:], op=ALU.subtract)
            nc.vector.scalar_tensor_tensor(
                out=PQ[:], in0=DQ[:], scalar=SCAL[:, 3:4], in1=PT[:],
                op0=ALU.mult, op1=ALU.add)

            YSD = wk.tile([64, GW], F32R)
            LCR = wk.tile([64, GW], F32R)
            MEGA = wk.tile([64, 20, GW], F32R)
            nc.vector.tensor_copy(YSD[:], PQ[64:128, :])
            nc.vector.tensor_scalar(out=LCR[:], in0=PQ[0:64, :], scalar1=SCAL[0:64, 2:3],
                                    scalar2=None, op0=ALU.mult)
            nc.vector.tensor_tensor(
                out=MEGA[:, 0:10, :],
                in0=PQ[0:64, :].unsqueeze(1).broadcast_to([64, 10, GW]),
                in1=WALLT[:, 0:10].unsqueeze(2).broadcast_to([64, 10, GW]),
                op=ALU.mult)
            nc.gpsimd.tensor_tensor(
                out=MEGA[:, 10:20, :],
                in0=PQ[0:64, :].unsqueeze(1).broadcast_to([64, 10, GW]),
                in1=WALLT[:, 10:20].unsqueeze(2).broadcast_to([64, 10, GW]),
                op=ALU.mult)

            # ---------------- per-level matmuls + extraction ----------------
            # transposed layout: out[y, ., x];  lhsT = YSD (stationary)
            POS = wk.tile([128, G], BF)
            PVA = wk.tile([128, 5, G], I32)

            for lvl in range(3):
                W = LVLW[lvl]
                xs = slice(LVLXO[lvl], LVLXO[lvl] + W)
                cps = psp.tile([W, W], F32, tag="cps", name="cps")
                sps = psp.tile([W, 5, 4, W], F32, tag="sps", name="sps")
                nc.tensor.matmul(cps[:], YSD[:, xs], LCR[:, xs], start=True, stop=True)
                if lvl < 2:
                    for pay in range(5):
                        nc.tensor.matmul(
                            sps[:, pay, :, :],
                            YSD[:, xs],
                            MEGA[:, 4 * pay:4 * pay + 4, xs],
                            start=True, stop=True)
                else:
                    for p2 in range(2):
                        nc.tensor.matmul(
                            sps[:, 2 * p2:2 * p2 + 2, :, :],
                            YSD[:, xs],
                            MEGA[:, 8 * p2:8 * p2 + 8, xs],
                            start=True, stop=True)
                    nc.tensor.matmul(
                        sps[:, 4, :, :], YSD[:, xs],
                        MEGA[:, 16:20, xs],
                        start=True, stop=True)

                if lvl == 0:
                    posl = POS[:, 0:128]
                else:
                    posl_t = wk.tile([W, W], BF, tag=f"posl{lvl}", name=f"posl{lvl}")
                    posl = posl_t[:]
                nc.scalar.sign(posl, cps[:])
                # m0 extraction: int bit-chain on DVE, is_ge range masks on Pool
                EI = wk.tile([W, W], I32, tag=f"ei{lvl}", name=f"ei{lvl}")
                M0F = wk.tile([W, W], F32, tag=f"m0f{lvl}", name=f"m0f{lvl}")
                I0 = wk.tile([W, W], I32, tag=f"i0{lvl}", name=f"i0{lvl}")
                SCB = wk.tile([W, W], I32, tag=f"scb{lvl}", name=f"scb{lvl}")
                nc.vector.tensor_scalar(out=EI[:], in0=cps[:].bitcast(I32),
                                        scalar1=23, scalar2=None, op0=ALU.arith_shift_right)
                nc.vector.tensor_scalar(out=M0F[:], in0=EI[:], scalar1=-0.5, scalar2=63.5,
                                        op0=ALU.mult, op1=ALU.add)
                nc.vector.tensor_copy(I0[:], M0F[:])
                nc.vector.tensor_scalar(out=I0[:], in0=I0[:], scalar1=7, scalar2=None,
                                        op0=ALU.bitwise_and)
                nc.vector.tensor_scalar(out=SCB[:], in0=I0[:], scalar1=27, scalar2=None,
                                        op0=ALU.logical_shift_left)
                nc.vector.tensor_scalar(out=SCB[:], in0=SCB[:], scalar1=127 << 23, scalar2=None,
                                        op0=ALU.add)
                MG8 = wk.tile([W, W], I32, tag=f"mg8{lvl}", name=f"mg8{lvl}")
                MG16 = wk.tile([W, W], I32, tag=f"mg16{lvl}", name=f"mg16{lvl}")
                MG24 = wk.tile([W, W], I32, tag=f"mg24{lvl}", name=f"mg24{lvl}")
                nc.vector.tensor_scalar(out=MG8[:], in0=M0F[:], scalar1=8.0, scalar2=None, op0=ALU.is_ge)
                nc.vector.tensor_scalar(out=MG16[:], in0=M0F[:], scalar1=16.0, scalar2=None, op0=ALU.is_ge)
                nc.vector.tensor_scalar(out=MG24[:], in0=M0F[:], scalar1=24.0, scalar2=None, op0=ALU.is_ge)
                TSEL = wk.tile([W, 5, W + 4], F32, tag=f"tsel{lvl}", name=f"tsel{lvl}")
                tsl = TSEL[:, :, 0:W]
                nc.scalar.copy(tsl, sps[:, :, 0, :])
                nc.vector.copy_predicated(tsl, MG8[:].unsqueeze(1).broadcast_to([W, 5, W]), sps[:, :, 1, :])
                nc.vector.copy_predicated(tsl, MG16[:].unsqueeze(1).broadcast_to([W, 5, W]), sps[:, :, 2, :])
                nc.vector.copy_predicated(tsl, MG24[:].unsqueeze(1).broadcast_to([W, 5, W]), sps[:, :, 3, :])
                nc.vector.tensor_tensor(
                    out=tsl, in0=tsl,
                    in1=SCB[:].bitcast(F32).unsqueeze(1).broadcast_to([W, 5, W]),
                    op=ALU.mult)
                GI = wk.tile([W, 5, W], I32, tag=f"gi{lvl}", name=f"gi{lvl}")
                if lvl == 0:
                    nc.vector.tensor_copy(PVA[:, :, 0:128], tsl)
                elif lvl == 1:
                    nc.vector.tensor_copy(GI[:], tsl)
                    gv = GI[:].rearrange("p q (g two) -> p q two g", two=2)
                    pv = posl.rearrange("p (g two) -> p two g", two=2)
                    nc.scalar.copy(PVA[0:64, :, 128:160], gv[:, :, 0, :])
                    nc.scalar.copy(PVA[64:128, :, 128:160], gv[:, :, 1, :])
                    nc.scalar.copy(POS[0:64, 128:160], pv[:, 0, :])
                    nc.scalar.copy(POS[64:128, 128:160], pv[:, 1, :])
                else:
                    nc.vector.tensor_copy(GI[:], tsl)
                    gv = GI[:].rearrange("p q (g four) -> p q four g", four=4)
                    pv = posl.rearrange("p (g four) -> p four g", four=4)
                    for j in range(4):
                        nc.gpsimd.tensor_copy(PVA[32 * j:32 * j + 32, :, 160:168], gv[:, :, j, :])
                        nc.gpsimd.tensor_copy(POS[32 * j:32 * j + 32, 160:168], pv[:, j, :])

            # ---------------- per-point targets (bf16, on Pool) ----------------
            TGT = wk.tile([128, 4, G], BF)
            nc.vector.scalar_tensor_tensor(
                out=TGT[:, 0:2, :], in0=PVA[:, 0:2, :], scalar=-0.03125, in1=XSYS,
                op0=ALU.mult, op1=ALU.add)
            nc.vector.scalar_tensor_tensor(
                out=TGT[:, 2:4, :], in0=PVA[:, 2:4, :], scalar=0.03125, in1=XSYS,
                op0=ALU.mult, op1=ALU.subtract)

            ACC = wk.tile([128, 8], F32)
            nc.gpsimd.memset(ACC[:], 0.0)

            # ---------------- dense focal (bf16) ----------------
            SGN = wk.tile([128, NCLS, G], BF)
            SP = wk.tile([128, NCLS, G], BF)
            SQ = wk.tile([128, NCLS, G], BF)
            BW = wk.tile([128, NCLS, G], BF)
            i_sgn = nc.scalar.activation(SGN[:], CLS[:], AF.Sigmoid, scale=-1.0)
            # centerness bce sigmoid (phase 1, input ready early)
            SPC = wk.tile([128, G], F32)
            i_bcesig = nc.scalar.activation(SPC[:], CTRP, AF.Sigmoid, scale=-1.0)
            add_dep_helper(i_bcesig.ins, i_sgn.ins, sync=False, reason="act order")
            # ---- ln phase 1 ----
            i_spln = nc.scalar.activation(SP[:], SGN[:], AF.Ln)      # -softplus(x)
            add_dep_helper(i_spln.ins, i_bcesig.ins, sync=False, reason="act order")
            i_sq = nc.scalar.activation(SQ[:], SGN[:], AF.Square, scale=-1.0, bias=1.0)
            i_bceln = nc.scalar.activation(SPC[:], SPC[:], AF.Ln)    # -softplus(ctr)
            BASE = SQ  # reuse buffer: BASE = SP * SQ
            nc.vector.tensor_tensor(out=BASE[:], in0=SP[:], in1=SQ[:], op=ALU.mult)

            # max over classes -> w-mask in logit space (Pool tree)
            MT10 = wk.tile([128, 10, G], BF)
            MT5 = wk.tile([128, 5, G], BF)
            MXL = wk.tile([128, G], BF)
            nc.vector.tensor_tensor(out=MT10[:], in0=CLS[:, 0:10, :], in1=CLS[:, 10:20, :], op=ALU.max)
            nc.vector.tensor_tensor(out=MT5[:], in0=MT10[:, 0:5, :], in1=MT10[:, 5:10, :], op=ALU.max)
            nc.vector.tensor_tensor(out=MT10[:, 0:2, :], in0=MT5[:, 0:2, :], in1=MT5[:, 2:4, :], op=ALU.max)
            nc.vector.tensor_tensor(out=MT10[:, 2:3, :], in0=MT10[:, 0:1, :], in1=MT10[:, 1:2, :], op=ALU.max)
            nc.vector.tensor_tensor(out=MXL[:].unsqueeze(1), in0=MT10[:, 2:3, :], in1=MT5[:, 4:5, :], op=ALU.max)
            HIM = wk.tile([128, G], BF)
            WBAR = wk.tile([128, G], BF)
            W16 = wk.tile([128, G], BF)
            nc.vector.tensor_scalar(out=HIM[:], in0=MXL[:], scalar1=LOGIT03, scalar2=None, op0=ALU.is_gt)
            nc.vector.tensor_scalar(out=WBAR[:], in0=POS[:], scalar1=-1.0, scalar2=1.0,
                                    op0=ALU.mult, op1=ALU.add)
            nc.vector.tensor_tensor(out=WBAR[:], in0=WBAR[:], in1=HIM[:], op=ALU.mult)
            nc.vector.tensor_scalar(out=W16[:], in0=WBAR[:], scalar1=-0.75, scalar2=0.75,
                                    op0=ALU.mult, op1=ALU.add)   # 0.75*w
            nc.vector.tensor_tensor(out=BW[:], in0=BASE[:],
                                    in1=W16[:].unsqueeze(1).broadcast_to([128, NCLS, G]),
                                    op=ALU.mult)                 # -base*w
            # PE: sum over all elements of -base*w
            ABP = psp.tile([1, 512], F32, tag="abp", name="abp")
            bw = BW[:].rearrange("p c g -> p (c g)")
            for i in range(7):
                n0 = i * 512
                n1 = min(n0 + 512, NCLS * G)
                nc.tensor.matmul(ABP[0:1, 0:n1 - n0], CST[:, 608:609].bitcast(BF)[:, 0:1],
                                 bw[:, n0:n1], start=(i == 0), stop=(i == 6))
            ABSB = wk.tile([1, 512], F32)
            i_absb = nc.scalar.copy(ABSB[:], ABP[:])
            nc.vector.tensor_reduce(out=ACC[0:1, 6:7], in_=ABSB[:], axis=AX.X, op=ALU.add)

            # ---------------- label-column logit ----------------
            LAB16 = wk.tile([128, G], BF)
            nc.vector.tensor_scalar(out=LAB16[:], in0=PVA[:, 4, :], scalar1=0.03125,
                                    scalar2=None, op0=ALU.mult)
            ISEQ = wk.tile([128, NCLS, G], BF)
            XLP = wk.tile([128, NCLS, G], BF)
            nc.vector.tensor_tensor(
                out=ISEQ[:], in0=LAB16[:].unsqueeze(1).broadcast_to([128, NCLS, G]),
                in1=IOTAX[:], op=ALU.is_equal)
            nc.vector.tensor_tensor(out=XLP[:], in0=ISEQ[:], in1=CLS[:], op=ALU.mult)
            XT10 = wk.tile([128, 10, G], BF)
            XT5 = wk.tile([128, 5, G], BF)
            XL = wk.tile([128, G], F32)
            nc.vector.tensor_tensor(out=XT10[:], in0=XLP[:, 0:10, :], in1=XLP[:, 10:20, :], op=ALU.add)
            nc.vector.tensor_tensor(out=XT5[:], in0=XT10[:, 0:5, :], in1=XT10[:, 5:10, :], op=ALU.add)
            nc.vector.tensor_tensor(out=XT10[:, 0:2, :], in0=XT5[:, 0:2, :], in1=XT5[:, 2:4, :], op=ALU.add)
            nc.vector.tensor_tensor(out=XT10[:, 2:3, :], in0=XT10[:, 0:1, :], in1=XT10[:, 1:2, :], op=ALU.add)
            nc.vector.tensor_tensor(out=XL[:].unsqueeze(1), in0=XT10[:, 2:3, :], in1=XT5[:, 4:5, :], op=ALU.add)
            # ---- sigmoid phase 2 ----
            SGL = wk.tile([128, G], F32)
            SGLN = wk.tile([128, G], F32)
            i_sgl = nc.scalar.activation(SGL[:], XL[:], AF.Sigmoid)
            i_sgln = nc.scalar.activation(SGLN[:], XL[:], AF.Sigmoid, scale=-1.0)
            add_dep_helper(i_sgl.ins, i_bceln.ins, sync=False, reason="act order")
            # ---- ln phase 2 ----
            SPL = wk.tile([128, G], F32)
            i_spl = nc.scalar.activation(SPL[:], SGL[:], AF.Ln)     # = -softplus(-xl)
            add_dep_helper(i_spl.ins, i_sgln.ins, sync=False, reason="act order")
            i_sgln2 = nc.scalar.activation(SGLN[:], SGLN[:], AF.Ln)  # = -softplus(xl)
            add_dep_helper(i_absb.ins, i_sgln2.ins, sync=False, reason="act tail order")
            B1 = wk.tile([128, G], F32)
            D1 = wk.tile([128, G], F32)
            D2 = wk.tile([128, G], F32)
            nc.vector.tensor_scalar(out=B1[:], in0=SGL[:], scalar1=-1.0, scalar2=1.0,
                                    op0=ALU.mult, op1=ALU.add)
            nc.gpsimd.tensor_tensor(out=B1[:], in0=B1[:], in1=B1[:], op=ALU.mult)
            nc.vector.scalar_tensor_tensor(out=D1[:], in0=SPL[:], scalar=-0.25, in1=B1[:],
                                           op0=ALU.mult, op1=ALU.mult)
            nc.gpsimd.tensor_tensor(out=SGL[:], in0=SGL[:], in1=SGL[:], op=ALU.mult)
            nc.vector.scalar_tensor_tensor(out=D2[:], in0=SGLN[:], scalar=-0.75, in1=SGL[:],
                                           op0=ALU.mult, op1=ALU.mult)
            nc.vector.tensor_tensor(out=D1[:], in0=D1[:], in1=D2[:], op=ALU.subtract)
            nc.vector.scalar_tensor_tensor(out=D2[:], in0=D1[:], scalar=1.0, in1=POS[:],
                                           op0=ALU.mult, op1=ALU.mult, accum_out=ACC[:, 5:6])

            # ---------------- GIoU (raw, bf16, clamped denominators) ----------
            MINS = wk.tile([128, 4, G], BF)
            MAXS = wk.tile([128, 4, G], BF)
            nc.vector.tensor_tensor(out=MINS[:], in0=REG, in1=TGT[:], op=ALU.min)
            nc.vector.tensor_tensor(out=MAXS[:], in0=REG, in1=TGT[:], op=ALU.max)
            SUMP = wk.tile([128, 2, G], BF)
            SUMT = wk.tile([128, 2, G], BF)
            WIHI = wk.tile([128, 2, G], BF)
            GWGH = wk.tile([128, 2, G], BF)
            nc.vector.tensor_tensor(out=SUMP[:], in0=REG[:, 0:2, :], in1=REG[:, 2:4, :], op=ALU.add)
            nc.gpsimd.tensor_tensor(out=SUMT[:], in0=TGT[:, 0:2, :], in1=TGT[:, 2:4, :], op=ALU.add)
            nc.vector.tensor_tensor(out=WIHI[:], in0=MINS[:, 0:2, :], in1=MINS[:, 2:4, :], op=ALU.add)
            nc.vector.tensor_tensor(out=GWGH[:], in0=MAXS[:, 0:2, :], in1=MAXS[:, 2:4, :], op=ALU.add)
            PAREA = wk.tile([128, G], BF)
            TAREA = wk.tile([128, G], BF)
            AI = wk.tile([128, G], BF)
            ACX = wk.tile([128, G], BF)
            nc.vector.tensor_tensor(out=PAREA[:], in0=SUMP[:, 0, :], in1=SUMP[:, 1, :], op=ALU.mult)
            nc.vector.tensor_tensor(out=TAREA[:], in0=SUMT[:, 0, :], in1=SUMT[:, 1, :], op=ALU.mult)
            nc.vector.tensor_tensor(out=AI[:], in0=WIHI[:, 0, :], in1=WIHI[:, 1, :], op=ALU.mult)
            nc.vector.tensor_tensor(out=ACX[:], in0=GWGH[:, 0, :], in1=GWGH[:, 1, :], op=ALU.mult)
            AU = wk.tile([128, G], BF)
            AU1 = wk.tile([128, G], F32)
            nc.vector.tensor_tensor(out=AU[:], in0=TAREA[:], in1=PAREA[:], op=ALU.add)
            nc.vector.tensor_tensor(out=AU[:], in0=AU[:], in1=AI[:], op=ALU.subtract)
            nc.vector.tensor_scalar(out=AU1[:], in0=AU[:], scalar1=1.0, scalar2=None, op0=ALU.add)
            # clamp |au+1| away from 0 (sign-free: max with eps keeps finiteness)
            nc.vector.tensor_scalar(out=AU1[:], in0=AU1[:], scalar1=1e-10, scalar2=None, op0=ALU.max)
            RAU = wk.tile([128, G], F32)
            IOUS = wk.tile([128, G], F32)
            nc.vector.reciprocal(RAU[:], AU1[:])
            nc.vector.tensor_scalar(out=IOUS[:], in0=AI[:], scalar1=1.0, scalar2=None, op0=ALU.add)
            nc.vector.tensor_tensor(out=IOUS[:], in0=IOUS[:], in1=RAU[:], op=ALU.mult)
            ACXC = wk.tile([128, G], F32)
            RAC = wk.tile([128, G], F32)
            T3 = wk.tile([128, G], F32)
            nc.vector.tensor_scalar(out=ACXC[:], in0=ACX[:], scalar1=1e-10, scalar2=None, op0=ALU.max)
            nc.vector.reciprocal(RAC[:], ACXC[:])
            nc.vector.tensor_tensor(out=T3[:], in0=ACX[:], in1=AU[:], op=ALU.subtract)
            nc.gpsimd.tensor_tensor(out=T3[:], in0=T3[:], in1=RAC[:], op=ALU.mult)
            LB = wk.tile([128, G], F32)
            # lb = 1 - ious + t3 (the +1 folds into the final stt below)
            nc.gpsimd.tensor_tensor(out=LB[:], in0=T3[:], in1=IOUS[:], op=ALU.subtract)

            # centerness target from raw tgt: sqrt via rsqrt magic (no table)
            LRMIN = wk.tile([128, G], BF)
            LRMAX = wk.tile([128, G], BF)
            TBMIN = wk.tile([128, G], BF)
            TBMAX = wk.tile([128, G], BF)
            nc.vector.tensor_tensor(out=LRMIN[:], in0=TGT[:, 0, :], in1=TGT[:, 2, :], op=ALU.min)
            nc.vector.tensor_tensor(out=LRMAX[:], in0=TGT[:, 0, :], in1=TGT[:, 2, :], op=ALU.max)
            nc.vector.tensor_tensor(out=TBMIN[:], in0=TGT[:, 1, :], in1=TGT[:, 3, :], op=ALU.min)
            nc.vector.tensor_tensor(out=TBMAX[:], in0=TGT[:, 1, :], in1=TGT[:, 3, :], op=ALU.max)
            NUMR = wk.tile([128, G], F32)
            DEN = wk.tile([128, G], F32)
            Z = wk.tile([128, G], F32)
            nc.gpsimd.tensor_tensor(out=NUMR[:], in0=LRMIN[:], in1=TBMIN[:], op=ALU.mult)
            nc.gpsimd.tensor_tensor(out=DEN[:], in0=LRMAX[:], in1=TBMAX[:], op=ALU.mult)
            nc.vector.tensor_scalar(out=NUMR[:], in0=NUMR[:], scalar1=0.0, scalar2=None, op0=ALU.max)
            nc.vector.tensor_scalar(out=DEN[:], in0=DEN[:], scalar1=1e-12, scalar2=None, op0=ALU.max)
            nc.gpsimd.tensor_tensor(out=Z[:], in0=NUMR[:], in1=DEN[:], op=ALU.mult)
            nc.vector.tensor_scalar(out=Z[:], in0=Z[:], scalar1=1e-20, scalar2=None, op0=ALU.max)
            # ctr = numr * rsqrt(z) = sqrt(numr/den); one Newton step
            R0 = wk.tile([128, G], I32)
            R2 = wk.tile([128, G], F32)
            V = wk.tile([128, G], F32)
            CTRT = wk.tile([128, G], F32)
            nc.vector.tensor_scalar(out=R0[:], in0=Z[:].bitcast(I32), scalar1=1,
                                    scalar2=None, op0=ALU.logical_shift_right)
            nc.vector.tensor_scalar(out=R0[:], in0=R0[:], scalar1=-1, scalar2=0x5f3759df,
                                    op0=ALU.mult, op1=ALU.add)
            R0F = R0[:].bitcast(F32)
            nc.gpsimd.tensor_tensor(out=R2[:], in0=R0F, in1=R0F, op=ALU.mult)
            nc.gpsimd.tensor_tensor(out=R2[:], in0=R2[:], in1=Z[:], op=ALU.mult)
            nc.gpsimd.tensor_scalar(out=V[:], in0=R2[:], scalar1=-0.5, scalar2=1.5,
                                    op0=ALU.mult, op1=ALU.add)
            nc.gpsimd.tensor_tensor(out=V[:], in0=V[:], in1=R0F, op=ALU.mult)
            nc.gpsimd.tensor_tensor(out=CTRT[:], in0=NUMR[:], in1=V[:], op=ALU.mult)
            W2 = wk.tile([128, G], F32)
            nc.gpsimd.tensor_tensor(out=W2[:], in0=CTRT[:], in1=POS[:], op=ALU.mult)
            LBW = wk.tile([128, G], F32)
            nc.vector.scalar_tensor_tensor(out=LBW[:], in0=LB[:], scalar=1.0, in1=W2[:],
                                           op0=ALU.add, op1=ALU.mult, accum_out=ACC[:, 2:3])
            # centerness bce
            UC = wk.tile([128, G], F32)
            nc.vector.tensor_tensor(out=UC[:], in0=CTRP, in1=CTRT[:], op=ALU.mult)
            nc.vector.tensor_tensor(out=UC[:], in0=SPC[:], in1=UC[:], op=ALU.add)
            VCP = wk.tile([128, G], F32)
            nc.vector.scalar_tensor_tensor(out=VCP[:], in0=UC[:], scalar=-1.0, in1=POS[:],
                                           op0=ALU.mult, op1=ALU.mult, accum_out=ACC[:, 3:4])
            # num_pos
            PCP = wk.tile([128, G], F32)
            nc.vector.tensor_scalar(out=PCP[:], in0=POS[:], scalar1=1.0, scalar2=0.0,
                                    op0=ALU.mult, op1=ALU.add, accum_out=ACC[:, 4:5])

            # ---------------- finalize: ship raw per-partition accumulators;
            # the host does the 128-way column sum (cheaper than a matmul +
            # copy + tiny DMA on the critical tail)
            nc.sync.dma_start(out_d.ap(), ACC[:])

    nc.compile()
    _CACHE["nc"] = nc
    return nc


def make_in_map(cls_l, reg_l, ctr_l, boxes, labels):
    """Build one core's input map from per-image numpy arrays."""
    scal, wallt = _prep_image(boxes, labels)

    def xmaj(p):
        # [C, h, w] -> x-major flat [C, w*h]
        c = p.shape[0]
        return p.transpose(0, 2, 1).reshape(c, -1)

    cls_cat = np.concatenate([xmaj(p.reshape(NCLS, p.shape[-2], p.shape[-1]))
                              for p in cls_l], 1)
    reg_cat = np.concatenate([xmaj(p.reshape(4, p.shape[-2], p.shape[-1]))
                              for p in reg_l], 1)
    ctr_cat = np.concatenate([xmaj(p.reshape(1, p.shape[-2], p.shape[-1]))
                              for p in ctr_l], 1)
    # partition-major repack: [C, (g p)] -> [p, C, g]
    cls_pm = cls_cat.reshape(NCLS, G, 128).transpose(2, 0, 1)
    regc = np.concatenate([reg_cat, ctr_cat], 0)
    reg_pm = regc.reshape(5, G, 128).transpose(2, 0, 1)
    cst = np.zeros((128, 624), np.float32)
    cst[:, 0:224] = GRID_C
    cst[:, 224:560] = XSYS_C.reshape(128, 336)
    cst[:, 580:588] = scal
    cst[0:64, 588:608] = wallt
    ones2 = np.full((128, 2), 1.0, _BF16)
    cst[:, 608:609] = ones2.view(np.float32)
    cst[:, 616] = 63.5
    cst[:, 617] = -7.0
    cst[:, 618] = -15.0
    cst[:, 619] = -23.0
    return {
        "cls": np.ascontiguousarray(cls_pm).astype(_BF16),
        "iotax": IOTAX_C,
        "reg": np.ascontiguousarray(reg_pm).astype(_BF16),
        "cst": cst,
    }


def combine_partials(parts):
    """parts: [n_cores, 8] -> [3] losses."""
    s = np.asarray(parts, np.float64).sum(0)
    Cv, D, E, F, ABn = s[2], s[3], s[4], s[5], s[6]
    np_ = max(E, 1.0)
    return np.array([(-ABn + F) / np_, Cv / np_, D / np_], np.float32)


def kernel(cls0, cls1, cls2, reg0, reg1, reg2, ctr0, ctr1, ctr2, boxes, labels,
           _trace=False):
    nc = _build()
    B = np.asarray(boxes).shape[0]
    in_maps = []
    for i in range(B):
        in_maps.append(make_in_map(
            [np.asarray(cls0)[i], np.asarray(cls1)[i], np.asarray(cls2)[i]],
            [np.asarray(reg0)[i], np.asarray(reg1)[i], np.asarray(reg2)[i]],
            [np.asarray(ctr0)[i], np.asarray(ctr1)[i], np.asarray(ctr2)[i]],
            np.asarray(boxes)[i], np.asarray(labels)[i]))
    res = run_bass_kernel_spmd(nc, in_maps, core_ids=list(range(B)), trace=_trace)
    parts = [r["out"].astype(np.float64).sum(axis=0) for r in res.results]
    out = combine_partials(parts)
    if _trace:
        return out, res
    return out
